# revision 1
# baseline (speedup 1.0000x reference)
"""Multi-head self-attention Trainium2 kernel (8-core data-parallel over batch).

Layout strategy (per core = one batch element):
  - host pre-transposes x -> xT [H, S] and weights -> W^T [in, out], so every
    matmul contracts over the SBUF partition axis with zero on-chip transposes.
  - qT, kT computed as [out, seq]; v computed natural [seq, out] with a ones
    column appended per head (v_aug) so the ctx matmul also produces softmax
    denominators for free (M=65 output rows; row 64 = sum_k probs).
  - scores computed transposed [k, q]; mask+scale+exp fused into a single
    ScalarE activation per tile: exp(smt[k] * s + cmt[k]) where smt=mask*scale,
    cmt=(1-mask)*(-10000). Two heads per PSUM tile via PE row-tiling (K=64).
  - matmuls run in bf16 (1 row/cycle on the PE, half the DMA/SBUF footprint of
    f32r); the softmax-denominator tail (stage/sums/ind) stays in fp32 bits
    (f32r) to keep the final error comfortably under tolerance.
  - q/k projections for pair pr+1 are emitted in small slices interleaved into
    pair pr's kc loop, so the PE always has dense work queued in-order while
    ScalarE (the attention-loop pacer) drains the masked-exp chain.
  - input weight DMAs ride the Activation HWDGE queue, x/out the SP queue
    (2 queues in parallel); output tiles DMA directly from PSUM.
  - softmax denominators are staged at partitions {0,32,64,96}, restacked by 4
    small DMAs into sums[0:6]/sums[32:38] (even/odd head pairs), reciprocated
    with one Ln + one Exp over the padded [0:38] range, broadcast across the
    64 d-rows of each head with tiny indicator matmuls, and applied chunk 5
    first so the out projection can start immediately after.
"""

import sys

for _p in ("/opt/trn_rl_repo", "/root/.axon_site/_ro/trn_rl_repo"):
    if _p not in sys.path:
        sys.path.append(_p)

import numpy as np

import concourse.bass as bass
import concourse.tile as tile
from concourse import mybir
from concourse.bass_utils import run_bass_kernel_spmd

F32 = mybir.dt.float32
F32R = mybir.dt.float32r
BF16 = mybir.dt.bfloat16
EXP = mybir.ActivationFunctionType.Exp
LN = mybir.ActivationFunctionType.Ln

B, S, H = 8, 1024, 768
NH, HD = 12, 64
NIC = H // 128          # 6 contraction chunks of 128
NSB = S // 128          # 8 seq blocks of 128
NPAIR = NH // 2         # 6 head pairs
VW = NH * (HD + 1)      # 780: v_aug columns per k-chunk (65 per head)
SCALE = HD ** -0.5

import os as _os
# fresh NeuronCore state at NRT init: stale device state from prior runs was
# observed to silently corrupt results (rel err drifting 6e-3 -> 0.2 on
# identical binaries); harmless if the runtime is already initialized
_os.environ.setdefault("NEURON_RT_RESET_CORES", "1")
_MMDT_NAME = _os.environ.get("MM_DTYPE", "bf16")
MMDT = {"f32": F32, "f32r": F32R, "bf16": BF16}[_MMDT_NAME]


def _split_excess_waits(nc, max_waits=1):
    """The pinned walrus rejects >1 semaphore wait per instruction
    ("Too many sync wait commands"). Waits are pre-conditions, so move the
    excess onto NOPs inserted immediately before the instruction. To keep the
    NOP count down, first elide waits already implied by an earlier >= wait on
    the same (engine, sem) — engines execute in program order, so a prior
    sem-ge-imm guarantee stays true (no mid-block sem resets outside the
    barrier instructions, which clear the tracking state below)."""
    for f in nc.m.functions:
        for bb in f.blocks:
            new_insts = []
            guaranteed = {}  # (engine, sem_id) -> max value already waited for
            for inst in bb.instructions:
                itype = type(inst).__name__
                if itype in ("InstEventSemaphore", "InstDrain",
                             "InstUnconditionalBranch", "InstConditionalBranch"):
                    guaranteed.clear()
                w = inst.sync_info.on_wait if inst.sync_info else None
                if w:
                    kept = []
                    for sw in w:
                        if sw.wait_mode == "sem-ge-imm" and sw.wait_value is not None:
                            key = (inst.engine, sw.id)
                            if guaranteed.get(key, -1) >= sw.wait_value:
                                continue
                            guaranteed[key] = sw.wait_value
                        kept.append(sw)
                    if not kept:
                        # keep at least one wait so sync_info stays valid
                        kept = [w[-1]]
                    if len(kept) > max_waits:
                        chunks = [kept[i:i + max_waits]
                                  for i in range(0, len(kept), max_waits)]
                        for ci, chunk in enumerate(chunks[:-1]):
                            new_insts.append(mybir.InstNoOp(
                                name=f"{inst.name}_waitsplit_{ci}",
                                engine=inst.engine,
                                sync_info=mybir.SyncInfo(
                                    on_wait=list(chunk), on_update=[]),
                                bass_nofuse=True,
                            ))
                        kept = chunks[-1]
                    inst.sync_info.on_wait = list(kept)
                new_insts.append(inst)
            bb.instructions[:] = new_insts


def _emit(ctx, tc, nc, d, with_bq, with_bk, with_bv):
    ts = bass.ts

    p_w = ctx.enter_context(tc.tile_pool(name="w", bufs=4))
    p_x = ctx.enter_context(tc.tile_pool(name="x", bufs=1))
    p_v = ctx.enter_context(tc.tile_pool(name="v", bufs=1))
    p_qk = ctx.enter_context(tc.tile_pool(name="qk", bufs=4))
    p_small = ctx.enter_context(tc.tile_pool(name="small", bufs=1))
    p_probs = ctx.enter_context(tc.tile_pool(name="probs", bufs=16))
    p_ctx = ctx.enter_context(tc.tile_pool(name="ctx", bufs=1))
    p_out = ctx.enter_context(tc.tile_pool(name="out", bufs=2))
    ps_main = ctx.enter_context(tc.tile_pool(name="psmain", bufs=3, space="PSUM"))
    ps_proj = ctx.enter_context(tc.tile_pool(name="psproj", bufs=1, space="PSUM"))
    ps_ctx = ctx.enter_context(tc.tile_pool(name="psctx", bufs=2, space="PSUM"))

    # ---- input DMAs: weights on the ACT hwdge queue, x on the SP queue ----
    xT_t = p_x.tile([128, NIC * S], MMDT, tag="x")
    w_tiles = {}

    def w_tile(wname):
        wt = p_w.tile([128, NIC * H], MMDT, tag="w", name=wname)
        w_tiles[wname] = wt
        return wt

    # Queue discipline: the ACT HWDGE queue must stay EMPTY before the exp
    # chain (queued transfers stall exp dispatch), so the head uses only the
    # SP queue (latency-critical small loads) and the idle GPSIMD SWDGE
    # queue (bulk weights).
    #
    # wk/wq arrive host-packed per head pair (row (pr ic p), col c), so one
    # contiguous 2D DMA per pair; SBUF layout [128, pr*768 + ic*128 + c].
    def load_w_pair(wname, pr, eng):
        eng.dma_start(
            out=w_tiles[wname][:, pr * H: (pr + 1) * H],
            in_=d[wname][pr * 128:(pr + 1) * 128, :])

    def load_w(wname, eng):
        # whole tensor as one 3D DMA: DRAM rows (ic p) -> SBUF column chunks
        wt = w_tile(wname)
        eng.dma_start(
            out=wt.rearrange("p (ic h) -> p ic h", ic=NIC),
            in_=d[wname].rearrange("(ic p) h -> p ic h", p=128))

    # mask tables first (first exp needs them)
    smt_t = p_small.tile([128, NSB], F32, tag="smt")
    nc.sync.dma_start(out=smt_t, in_=d["smt"][:, :])
    cmt_t = p_small.tile([128, NSB], F32, tag="cmt")
    nc.sync.dma_start(out=cmt_t, in_=d["cmt"][:, :])
    # preload the exp/ln spline table while the pipeline is still DMA-bound,
    # so the first real exp doesn't pay the ~1.3us ACT_TABLE_LOAD
    warm_t = p_small.tile([1, 1], F32, tag="warm")
    nc.scalar.activation(out=warm_t, in_=smt_t[0:1, 0:1], func=EXP)

    # All input DMAs ride the SP queue: the ACT HWDGE queue stalls exp
    # dispatch behind its transfers on real hardware (measured +130us), and
    # GPSIMD software-DGE descriptor prep is slow. Order = earliest consumer:
    # wv before x so the kc=0 v-proj hook never stalls, pair-0 wk/wq before
    # the x halves that pace the first projection.
    w_tile("wkT")
    w_tile("wqT")
    load_w_pair("wkT", 0, nc.sync)
    load_w_pair("wqT", 0, nc.sync)
    for ic in range(NIC):
        nc.sync.dma_start(out=xT_t[:, ic * S: ic * S + 512],
                          in_=d["xT"][ts(ic, 128), 0:512])
    load_w("wvT", nc.gpsimd)
    for pr in range(1, NPAIR):
        load_w_pair("wkT", pr, nc.gpsimd)
        load_w_pair("wqT", pr, nc.gpsimd)
    load_w("woT", nc.gpsimd)
    for ic in range(NIC):
        nc.sync.dma_start(out=xT_t[:, ic * S + 512: (ic + 1) * S],
                          in_=d["xT"][ts(ic, 128), 512:1024])
    ind_t = p_small.tile([64, H], F32R, tag="ind")
    nc.sync.dma_start(out=ind_t, in_=d["ind"][:, :])
    bias_tiles = {}
    for bname, flag in (("bqT", with_bq), ("bkT", with_bk)):
        if flag:
            bt = p_small.tile([128, NIC], F32, tag=bname)
            nc.sync.dma_start(out=bt, in_=d[bname][:, :])
            bias_tiles[bname] = bt

    # per-head softmax denominators: compute engines can only write start
    # partitions in {0,32,64,96}, so pack head (pair pr, hh) sums on
    # partition 32*(2*(pr%2)+hh), column block pr//2; 4 small DMAs restack to
    # sums_t[3*hh+b] (even pairs) / sums_t[32+3*hh+b] (odd pairs).
    stage_t = p_small.tile([128, (NPAIR // 2) * S], F32R, tag="stage")
    sums_t = p_small.tile([64, S], F32R, tag="sums")
    # rows 6..31 inside the padded Ln/Exp partition ranges hold whatever was
    # in SBUF — the activations produce junk there, but those rows are never
    # read (the bc matmuls only contract rows 0:6 / 32:38)

    # ---- v projection -> v_aug [128, kc*VW + h*65 + d], col 64 of head = 1 ----
    v_t = p_v.tile([128, NSB * VW], MMDT, tag="v")
    v_view = v_t.rearrange("p (kc h e) -> p kc h e", kc=NSB, h=NH)
    nc.sync.dma_start(
        out=v_view[:, :, :, HD:HD + 1],
        in_=d["vones"][:, :].rearrange("p (kc h e) -> p kc h e", kc=NSB, h=NH))

    def v_proj_block(sb):
        wv = w_tiles["wvT"]
        for oh in range(2):
            o0, ow = (0, 512) if oh == 0 else (512, 256)
            ps = ps_main.tile([128, 512], F32, tag="mm", name=f"vp{sb}_{oh}")
            for ic in range(NIC):
                nc.tensor.matmul(
                    ps[:, 0:ow],
                    xT_t[:, ic * S + sb * 128: ic * S + (sb + 1) * 128],
                    wv[:, ic * H + o0: ic * H + o0 + ow],
                    start=(ic == 0), stop=(ic == NIC - 1),
                )
            h0, hn = (0, 8) if oh == 0 else (8, 4)
            src = ps[:, 0:ow].rearrange("p (h e) -> p h e", e=HD)
            dst_ap = v_view[:, sb, h0:h0 + hn, 0:HD]
            if with_bv:
                bv_b = d["bv_bc"]
                nc.vector.tensor_add(out=dst_ap, in0=src, in1=bv_b[:, sb, h0:h0 + hn, :])
            else:
                nc.vector.tensor_copy(out=dst_ap, in_=src)

    # ---- q/k projection, emitted in slices so it interleaves with attention.
    # Each pair needs 4 psum groups (k/q x seq-half); one group = 6 matmuls +
    # 1 copy, all through the single-bank ps_proj pool. A group's matmuls are
    # emitted consecutively (other PE work between groups hides the copy).
    def make_proj_slices(pr):
        qk = {
            "wkT": p_qk.tile([128, S], MMDT, tag="qk", name=f"k_b{pr}"),
            "wqT": p_qk.tile([128, S], MMDT, tag="qk", name=f"q_b{pr}"),
        }

        def group(wname, bname, flag, sh):
            def emit():
                wt = w_tiles[wname]
                dst = qk[wname]
                ps = ps_proj.tile([128, 512], F32, tag="pj", name=f"pj{pr}{wname}{sh}")
                for ic in range(NIC):
                    nc.tensor.matmul(
                        ps,
                        wt[:, pr * H + ic * 128: pr * H + (ic + 1) * 128],
                        xT_t[:, ic * S + sh * 512: ic * S + (sh + 1) * 512],
                        start=(ic == 0), stop=(ic == NIC - 1),
                    )
                dst_ap = dst[:, sh * 512:(sh + 1) * 512]
                if flag:
                    nc.vector.tensor_scalar_add(
                        out=dst_ap, in0=ps, scalar1=bias_tiles[bname][:, pr:pr + 1])
                else:
                    nc.vector.tensor_copy(out=dst_ap, in_=ps)
            return emit

        # sh=0 groups first: scores for kc 0..3 / qh=0 only touch the first
        # 512 columns of kT/qT, so attention can start before sh=1 lands
        slices = [
            group("wkT", "bkT", with_bk, 0),
            group("wqT", "bqT", with_bq, 0),
            group("wkT", "bkT", with_bk, 1),
            group("wqT", "bqT", with_bq, 1),
        ]
        return qk["wkT"], qk["wqT"], slices

    ctx_t = p_ctx.tile([128, NIC * S], MMDT, tag="ctxT")

    # ---- attention for one pair; hooks[kc] emits extra PE work (next pair's
    # projection slices / v-proj blocks) between scores and lagged ctx ----
    def attention(pr, kT_p, qT_p, hooks):
        ctxA = ps_ctx.tile([HD + 1, S], F32, tag="ctx", name=f"cA{pr}")
        ctxB = ps_ctx.tile([HD + 1, S], F32, tag="ctx", name=f"cB{pr}")
        cps = (ctxA, ctxB)
        pending = []  # (kc, qh, hh, probs_tile) awaiting ctx matmul
        LAG = 2       # kc distance between exp and its ctx matmul
        for kc in range(NSB):
            for qh in range(2):
                for hh in range(2):
                    sc = ps_main.tile([128, 512], F32, tag="mm",
                                      name=f"sc{pr}_{kc}_{qh}_{hh}")
                    nc.tensor.matmul(
                        sc,
                        kT_p[hh * 64:(hh + 1) * 64, kc * 128:(kc + 1) * 128],
                        qT_p[hh * 64:(hh + 1) * 64, qh * 512:(qh + 1) * 512],
                        start=True, stop=True, tile_position=(hh * 64, 0),
                    )
                    probs = p_probs.tile([128, 512], MMDT, tag="probs",
                                         name=f"pb{pr}_{kc}_{qh}_{hh}")
                    nc.scalar.activation(
                        out=probs, in_=sc, func=EXP,
                        scale=smt_t[:, kc:kc + 1], bias=cmt_t[:, kc:kc + 1],
                    )
                    pending.append((kc, qh, hh, probs))
            for fn in hooks[kc]:
                fn()
            # lag the ctx matmuls LAG kcs behind their probs, so the PE has
            # scores/proj work queued while ScalarE produces the exp tiles
            while pending and pending[0][0] <= kc - LAG:
                kcp, qh, hh, probs = pending.pop(0)
                voff = kcp * VW + (2 * pr + hh) * (HD + 1)
                nc.tensor.matmul(
                    cps[hh][:, qh * 512:(qh + 1) * 512],
                    v_t[:, voff: voff + HD + 1],
                    probs,
                    start=(kcp == 0), stop=(kcp == NSB - 1),
                )
        for (kcp, qh, hh, probs) in pending:
            voff = kcp * VW + (2 * pr + hh) * (HD + 1)
            nc.tensor.matmul(
                cps[hh][:, qh * 512:(qh + 1) * 512],
                v_t[:, voff: voff + HD + 1],
                probs,
                start=(kcp == 0), stop=(kcp == NSB - 1),
            )
        # stage (denominator) copies first: they gate the restack DMAs, while
        # the big ctx copies only gate the (later) normalize multiplies. On
        # the last pair ScalarE is already drained, so it takes one stage copy
        # and both restack chains start a DVE-copy earlier.
        for hh, ctx_ps in ((0, ctxA), (1, ctxB)):
            sp = 32 * (2 * (pr % 2) + hh)
            eng = nc.scalar if (hh == 1 and pr == NPAIR - 1) else nc.vector
            eng_copy = (nc.scalar.copy if eng is nc.scalar
                        else nc.vector.tensor_copy)
            eng_copy(
                out=stage_t[sp:sp + 1, (pr // 2) * S:(pr // 2 + 1) * S],
                in_=ctx_ps[HD:HD + 1, :],
            )
        for hh, ctx_ps in ((0, ctxA), (1, ctxB)):
            nc.vector.tensor_copy(
                out=ctx_t[hh * 64:(hh + 1) * 64, pr * S:(pr + 1) * S],
                in_=ctx_ps[0:HD, :],
            )

    def restack(i, eng):
        # stage partition 32*i -> sums rows base+3*(i%2)..+3 (see layout note)
        base = 0 if i < 2 else 32
        r0 = base + 3 * (i % 2)
        eng.dma_start(
            out=sums_t[r0:r0 + 3, :],
            in_=stage_t[32 * i:32 * i + 1, :])

    # normalize one ctxT chunk: broadcast the head recips across the 64
    # d-rows with a tiny indicator matmul, then one elementwise multiply
    def norm_chunk(ic, pool):
        base = 0 if ic % 2 == 0 else 32
        for qh in range(2):
            bc = pool.tile([128, 512], F32, tag="pj", name=f"bc{ic}_{qh}")
            nc.tensor.matmul(
                bc,
                ind_t[base:base + 6, ts(ic, 128)],
                sums_t[base:base + 6, qh * 512:(qh + 1) * 512],
                start=True, stop=True,
            )
            cslice = ctx_t[:, ic * S + qh * 512: ic * S + (qh + 1) * 512]
            nc.vector.tensor_mul(out=cslice, in0=cslice, in1=bc)

    # ---- pipeline: pair 0's projection up front; v-proj block sb lands at
    # pair 0's kc=sb hook (one iteration before ctx needs chunk sb, thanks to
    # the ctx lag), so ScalarE starts ~15us earlier than a serial v phase.
    # Later pairs interleave pair pr+1's projection slices at kc 1,3,4,6. ----
    kT0, qT0, slices0 = make_proj_slices(0)
    for emit in slices0:
        emit()

    cur = (kT0, qT0)
    for pr in range(NPAIR):
        hooks = [[] for _ in range(NSB)]
        if pr == 0:
            for sb in range(NSB):
                hooks[sb].append(lambda sb=sb: v_proj_block(sb))
        if pr + 1 < NPAIR:
            kTn, qTn, slices = make_proj_slices(pr + 1)
            for i, kc in enumerate((1, 3, 4, 6)):
                hooks[kc].append(slices[i])
        else:
            # final pair: the even pairs' denominators are staged and
            # restacked, so reciprocate + normalize chunks 0/2/4 in the slack
            # (ps_proj is idle — no next projection)
            def early_recip():
                nc.scalar.activation(out=sums_t[0:6, :], in_=sums_t[0:6, :],
                                     func=LN)
                nc.scalar.activation(out=sums_t[0:6, :], in_=sums_t[0:6, :],
                                     func=EXP, scale=-1.0)
            hooks[1].append(early_recip)
            hooks[3].append(lambda: norm_chunk(0, ps_proj))
            hooks[5].append(lambda: norm_chunk(2, ps_proj))
            hooks[7].append(lambda: norm_chunk(4, ps_proj))
        attention(pr, cur[0], cur[1], hooks)
        if pr + 1 < NPAIR:
            cur = (kTn, qTn)
        if pr == NPAIR - 2:
            restack(0, nc.sync)
            restack(1, nc.sync)

    # ---- tail: odd pairs' reciprocals in column halves (left lands first and
    # unblocks the left-half normalize + out-proj chain), while the out
    # projection already streams the (ready) even chunks ----
    # stage row 64 -> sums rows 32..34, row 96 -> rows 35..37, split by half
    for half, h0 in enumerate((0, 512)):
        eng_a, eng_b = (nc.sync, nc.scalar)
        in_a = stage_t[64:65, :].rearrange("p (b c) -> p b c", b=3)[:, :, h0:h0 + 512]
        eng_a.dma_start(out=sums_t[32:35, h0:h0 + 512], in_=in_a)
        in_b = stage_t[96:97, :].rearrange("p (b c) -> p b c", b=3)[:, :, h0:h0 + 512]
        eng_b.dma_start(out=sums_t[35:38, h0:h0 + 512], in_=in_b)
        nc.scalar.activation(out=sums_t[32:38, h0:h0 + 512],
                             in_=sums_t[32:38, h0:h0 + 512], func=LN)
        nc.scalar.activation(out=sums_t[32:38, h0:h0 + 512],
                             in_=sums_t[32:38, h0:h0 + 512], func=EXP,
                             scale=-1.0)

    wo = w_tiles["woT"]
    op_state = {}

    def op_matmuls(sb, jh, ic, pool):
        key = (sb, jh)
        j0, jw = (0, 512) if jh == 0 else (512, 256)
        if key not in op_state:
            op_state[key] = pool.tile([128, 512], F32, tag="mm" if pool is ps_main else "ctx",
                                      name=f"op{sb}_{jh}")
        ps = op_state[key]
        nc.tensor.matmul(
            ps[:, 0:jw],
            ctx_t[:, ic * S + sb * 128: ic * S + (sb + 1) * 128],
            wo[:, ic * H + j0: ic * H + j0 + jw],
            start=(ic == 0), stop=(ic == 5))

    def op_pool(sb):
        return ps_main if sb % 2 == 0 else ps_ctx

    def op_finish(sb, ot):
        for jh in range(2):
            for ic in (1, 3, 5):
                op_matmuls(sb, jh, ic, op_pool(sb))
            j0, jw = (0, 512) if jh == 0 else (512, 256)
            nc.scalar.copy(out=ot[:, j0:j0 + jw],
                           in_=op_state.pop((sb, jh))[:, 0:jw])
        nc.sync.dma_start(out=d["out"][ts(sb, 128), :], in_=ot)

    # even chunks were normalized during pair 5, so sb0/sb1 start their
    # accumulation immediately (keeping the PE warm while the odd-group
    # reciprocal chain runs); odd-chunk matmuls follow their bc matmuls in
    # PE program order to avoid a cross-engine ordering deadlock
    ot0 = p_out.tile([128, H], F32, tag="out", name="ot0")
    ot1 = p_out.tile([128, H], F32, tag="out", name="ot1")
    for sb in (0, 1):
        for jh in range(2):
            for ic in (0, 2, 4):
                op_matmuls(sb, jh, ic, op_pool(sb))
    for ic in (1, 3, 5):
        norm_chunk(ic, ps_proj)
    op_finish(0, ot0)
    op_finish(1, ot1)
    for sb in range(2, NSB):
        ot = p_out.tile([128, H], F32, tag="out")
        for jh in range(2):
            for ic in (0, 2, 4, 1, 3, 5):
                op_matmuls(sb, jh, ic, op_pool(sb))
            j0, jw = (0, 512) if jh == 0 else (512, 256)
            nc.scalar.copy(out=ot[:, j0:j0 + jw],
                           in_=op_state.pop((sb, jh))[:, 0:jw])
        nc.sync.dma_start(out=d["out"][ts(sb, 128), :], in_=ot)


def declare_params(nc, with_bq=False, with_bk=False, with_bv=False):
    d = {
        "xT": nc.declare_dram_parameter("xT", [H, S], MMDT, isOutput=False).ap(),
        "wqT": nc.declare_dram_parameter("wqT", [H, H], MMDT, isOutput=False).ap(),
        "wkT": nc.declare_dram_parameter("wkT", [H, H], MMDT, isOutput=False).ap(),
        "wvT": nc.declare_dram_parameter("wvT", [H, H], MMDT, isOutput=False).ap(),
        "woT": nc.declare_dram_parameter("woT", [H, H], MMDT, isOutput=False).ap(),
        "smt": nc.declare_dram_parameter("smt", [128, NSB], F32, isOutput=False).ap(),
        "cmt": nc.declare_dram_parameter("cmt", [128, NSB], F32, isOutput=False).ap(),
        "ind": nc.declare_dram_parameter("ind", [64, H], F32R, isOutput=False).ap(),
        "vones": nc.declare_dram_parameter("vones", [128, NSB * NH], MMDT, isOutput=False).ap(),
        "out": nc.declare_dram_parameter("out", [S, H], F32, isOutput=True).ap(),
    }
    if with_bq:
        d["bqT"] = nc.declare_dram_parameter("bqT", [128, NIC], F32, isOutput=False).ap()
    if with_bk:
        d["bkT"] = nc.declare_dram_parameter("bkT", [128, NIC], F32, isOutput=False).ap()
    if with_bv:
        bvb = nc.declare_dram_parameter("bv_bc", [128, NSB * NH * HD], F32, isOutput=False)
        d["bv_bc"] = bvb.ap().rearrange("p (kc h e) -> p kc h e", kc=NSB, h=NH)
    return d


def build_nc(with_bq=False, with_bk=False, with_bv=False, split_waits=True):
    nc = bass.Bass("TRN2", target_bir_lowering=False, debug=False)
    d = declare_params(nc, with_bq, with_bk, with_bv)

    from contextlib import ExitStack
    with tile.TileContext(nc) as tc, ExitStack() as es:
        _emit(es, tc, nc, d, with_bq, with_bk, with_bv)
    if split_waits:
        _split_excess_waits(nc)
    return nc


def make_in_maps(x, attention_mask, Wq, bq, Wk, bk, Wv, bv, Wo, bo):
    with_bq = bool(np.any(bq)) if bq is not None else False
    with_bk = bool(np.any(bk)) if bk is not None else False
    with_bv = bool(np.any(bv)) if bv is not None else False

    if _MMDT_NAME == "bf16":
        import ml_dtypes
        mm_np = ml_dtypes.bfloat16
    else:
        mm_np = np.float32
    def pack_pairs(W):
        # W^T [in, out] -> rows (pr ic p), cols c: each head pair's weights
        # contiguous, so the kernel loads a pair with one 2D DMA
        WT = np.asarray(W, np.float32).T.reshape(NIC, 128, NPAIR, 128)
        return np.ascontiguousarray(
            WT.transpose(2, 1, 0, 3).reshape(NPAIR * 128, H)).astype(mm_np)

    shared = {
        "wqT": pack_pairs(Wq),
        "wkT": pack_pairs(Wk),
        "wvT": np.ascontiguousarray(np.asarray(Wv, np.float32).T).astype(mm_np),
        "woT": np.ascontiguousarray(np.asarray(Wo, np.float32).T).astype(mm_np),
    }
    # head h = 2*pr+hh lives in sums row (32 if pr odd else 0) + 3*hh + pr//2
    # and in ctx_t partitions hh*64..hh*64+63 of column chunk pr.
    ind = np.zeros((64, H), np.float32)
    for pr in range(NPAIR):
        for hh in range(2):
            j = (0 if pr % 2 == 0 else 32) + 3 * hh + pr // 2
            ind[j, pr * 128 + hh * HD: pr * 128 + (hh + 1) * HD] = 1.0
    shared["ind"] = ind
    shared["vones"] = np.ones((128, NSB * NH), np.float32).astype(mm_np)
    if with_bq:
        shared["bqT"] = np.ascontiguousarray(
            np.asarray(bq, np.float32).reshape(NIC, 128).T)
    if with_bk:
        shared["bkT"] = np.ascontiguousarray(
            np.asarray(bk, np.float32).reshape(NIC, 128).T)
    if with_bv:
        # [128, kc*NH*HD] replicated bias in the v_aug head layout (minus ones col)
        bvv = np.asarray(bv, np.float32).reshape(NH, HD)
        bvb = np.broadcast_to(bvv[None, None], (128, NSB, NH, HD))
        shared["bv_bc"] = np.ascontiguousarray(bvb.reshape(128, NSB * NH * HD))

    x = np.asarray(x, np.float32)
    mask = np.asarray(attention_mask)
    in_maps = []
    for c in range(B):
        m = mask[c].astype(np.float32)
        smt = np.ascontiguousarray((m * SCALE).reshape(NSB, 128).T)
        cmt = np.ascontiguousarray(((1.0 - m) * -10000.0).reshape(NSB, 128).T)
        in_maps.append({
            "xT": np.ascontiguousarray(x[c].T).astype(mm_np),
            "smt": smt, "cmt": cmt, **shared,
        })
    return in_maps, (with_bq, with_bk, with_bv)


_nc_cache = {}


def kernel(x, attention_mask, Wq, bq, Wk, bk, Wv, bv, Wo, bo):
    in_maps, flags = make_in_maps(x, attention_mask, Wq, bq, Wk, bk, Wv, bv, Wo, bo)
    if flags not in _nc_cache:
        _nc_cache[flags] = build_nc(*flags)
    nc = _nc_cache[flags]
    res = run_bass_kernel_spmd(nc, in_maps, core_ids=list(range(B)))
    out = np.stack([res.results[c]["out"] for c in range(B)], axis=0)
    out = out + np.asarray(bo, np.float32)[None, None, :]
    return out.astype(np.float32)

